# revision 13
# baseline (speedup 1.0000x reference)
"""Trainium2 Bass kernel for nn_BasicTransformerBlock (sparse attention).

Strategy: data-parallel over the 16 sparse batches (kk, b) -> 2 per core
on 8 NeuronCores.  Each core runs QKV projection + 3D RoPE + per-head
attention + output projection for its 2 sparse sequences of 512 tokens.
Device math in bf16 with fp32 PSUM accumulation.

Self-contained: hardcodes shapes for s=2048, b=4, dim=2304 (24 heads x 96),
sparse n=4, frame/height/width = 8/16/16.
"""
import numpy as np
import ml_dtypes

import concourse.bass as bass
import concourse.tile as tile
import concourse.mybir as mybir
from concourse.bass_utils import run_bass_kernel_spmd
from concourse.masks import make_identity
from concourse.vector_clock import ScopedClock

F32 = mybir.dt.float32
BF16 = mybir.dt.bfloat16
AF = mybir.ActivationFunctionType

HEADS = 24
HD = 96
DIM = 2304
S = 2048
B = 4
NSP = 4          # sparse n
SPS = 512        # sparse seq len
NCORES = 8
TOK = 512        # tokens per pass (one sparse batch)
KT = DIM // 128  # 18 contraction tiles
MT = DIM // 128  # 18 output-dim tiles
ROPE_FREQ = 10000.0


class _TC(tile.TileContext):
    """This walrus build allows only one sync-wait per instruction; split
    multi-wait instructions into single-wait same-engine nops."""

    def _drain_and_barrier(self, tick_clock, wait_clock):
        nc = self.nc
        drain_inst = nc.sync.drain()
        wait_clock.add_sem_waits(
            drain_inst.ins, ScopedClock({None: tick_clock.global_clock})
        )
        nc.all_engine_barrier()
        assert self.sems is not None
        popped = nc._tile_sem_poison_stack.pop()
        assert popped is self._sem_poison
        nc.clear_and_free_semaphores(list(self.sems.allocated().values()))
        nc.all_engine_barrier()
        self._split_multi_waits()

    def _split_multi_waits(self):
        nc = self.nc
        for f in nc.m.functions:
            for bb in f.blocks:
                out = []
                for inst in bb.instructions:
                    si = getattr(inst, "sync_info", None)
                    waits = list(si.on_wait) if si and si.on_wait else []
                    if len(waits) > 1:
                        si.on_wait = waits[-1:]
                        eng = nc.engines[inst.engine]
                        cb = nc.cur_bb.bb
                        for w in waits[:-1]:
                            ni = eng.nop().ins
                            assert cb.instructions[-1] is ni
                            cb.instructions.pop()
                            ni.sync_info = mybir.SyncInfo(
                                on_wait=[w], on_update=[])
                            out.append(ni)
                    out.append(inst)
                bb.instructions[:] = out


def _allowance(x):
    # legal engine partition ranges: base 0 -> any, 64 -> <=64, 32/96 -> <=32
    if x == 0:
        return 128
    if x == 64:
        return 64
    if x in (32, 96):
        return 32
    return 32 - x % 32


def _legal_chunks(src0, dst0, length):
    """Split [0, length) so each chunk is a legal partition range at both
    src0+off and dst0+off."""
    out = []
    off = 0
    while off < length:
        step = min(_allowance(src0 + off), _allowance(dst0 + off),
                   length - off)
        out.append((off, step))
        off += step
    return out


def _head_segments(m):
    """For Q^T psum m-tile rows r (global outdim g=128m+r), yield
    (r0, len, h, d0): head h, head-dim start d0."""
    segs = []
    r = 0
    while r < 128:
        g = 128 * m + r
        h, d = divmod(g, HD)
        ln = min(128 - r, HD - d)
        for off, step in _legal_chunks(r, d, ln):
            segs.append((r + off, step, h, d + off))
        r += ln
    return segs


def build_nc():
    nc = bass.Bass("TRN2", target_bir_lowering=False, debug=False)

    x_d = nc.dram_tensor("x", [DIM, 2 * TOK], F32, kind="ExternalInput")
    w_d = {n: nc.dram_tensor(n, [DIM, DIM], F32, kind="ExternalInput")
           for n in ("wq", "wk", "wv", "wo")}
    cos_d = nc.dram_tensor("cos", [HD, 2 * TOK], BF16, kind="ExternalInput")
    sin_d = nc.dram_tensor("sin", [HD, 2 * TOK], BF16, kind="ExternalInput")
    bq_d = nc.dram_tensor("bq", [HD, HEADS], F32, kind="ExternalInput")
    bk_d = nc.dram_tensor("bk", [HD, HEADS], F32, kind="ExternalInput")
    bo_d = nc.dram_tensor("bo", [128, MT], F32, kind="ExternalInput")
    out_d = nc.dram_tensor("out", [DIM, 2 * TOK], F32, kind="ExternalOutput")

    scale = HD ** -0.5
    VN = [480, 480, 480, 480, 384]  # V out-dim chunks: 5,5,5,5,4 heads

    with _TC(nc) as tc:
        with tc.tile_pool(name="const", bufs=1) as cpool, \
             tc.tile_pool(name="xb", bufs=KT + 1) as xpool, \
             tc.tile_pool(name="qkt", bufs=1) as qkpool, \
             tc.tile_pool(name="vsb", bufs=4) as vpool, \
             tc.tile_pool(name="osb", bufs=4) as opool, \
             tc.tile_pool(name="wf", bufs=3) as wfpool, \
             tc.tile_pool(name="wb", bufs=3) as wbpool, \
             tc.tile_pool(name="wvf", bufs=4) as wvfpool, \
             tc.tile_pool(name="wvb", bufs=4) as wvbpool, \
             tc.tile_pool(name="xf", bufs=3) as xfpool, \
             tc.tile_pool(name="et", bufs=12) as epool, \
             tc.tile_pool(name="of", bufs=2) as ofpool, \
             tc.tile_pool(name="small", bufs=8) as spool, \
             tc.tile_pool(name="ps", bufs=6, space="PSUM") as psp, \
             tc.tile_pool(name="pst", bufs=2, space="PSUM") as ptp:

            ident = cpool.tile([128, 128], BF16)
            make_identity(nc, ident[:])
            cos_sb = cpool.tile([HD, 2 * TOK], BF16)
            nc.sync.dma_start(cos_sb[:], cos_d.ap())
            sin_sb = cpool.tile([HD, 2 * TOK], BF16)
            nc.sync.dma_start(sin_sb[:], sin_d.ap())
            bq_sb = cpool.tile([HD, HEADS], F32)
            nc.sync.dma_start(bq_sb[:], bq_d.ap())
            bk_sb = cpool.tile([HD, HEADS], F32)
            nc.sync.dma_start(bk_sb[:], bk_d.ap())
            bo_sb = cpool.tile([128, MT], F32)
            nc.sync.dma_start(bo_sb[:], bo_d.ap())

            for sb in range(2):
                # ---- load X^T (this pass's 512 tokens), cast to bf16 ----
                xb = []
                for kt in range(KT):
                    xf = xfpool.tile([128, TOK], F32)
                    nc.gpsimd.dma_start(
                        xf[:], x_d.ap()[kt * 128:(kt + 1) * 128,
                                        sb * TOK:(sb + 1) * TOK])
                    t = xpool.tile([128, TOK], BF16)
                    nc.any.tensor_copy(t[:], xf[:])
                    xb.append(t)

                # ---- Q^T / K^T projections with bias, into head-aligned
                #      (96, 24*512) layout ----
                qT = qkpool.tile([HD, HEADS * TOK], BF16, tag="qT")
                kT = qkpool.tile([HD, HEADS * TOK], BF16, tag="kT")
                for name, dst, bias in (("wq", qT, bq_sb), ("wk", kT, bk_sb)):
                    for m in range(MT):
                        wf = wfpool.tile([128, DIM], F32)
                        nc.sync.dma_start(
                            wf[:].rearrange("p (kt c) -> p kt c", kt=KT),
                            w_d[name].ap()[:, m * 128:(m + 1) * 128]
                            .rearrange("(kt p) c -> p kt c", p=128))
                        wb = wbpool.tile([128, DIM], BF16)
                        nc.any.tensor_copy(wb[:], wf[:])
                        ps = psp.tile([128, TOK], F32, tag="ps")
                        for kt in range(KT):
                            nc.tensor.matmul(
                                ps[:], wb[:, kt * 128:(kt + 1) * 128],
                                xb[kt][:], start=(kt == 0), stop=(kt == KT - 1))
                        for r0, ln, h, d0 in _head_segments(m):
                            nc.scalar.activation(
                                dst[d0:d0 + ln, h * TOK:(h + 1) * TOK],
                                ps[r0:r0 + ln, :], AF.Identity,
                                bias=bias[d0:d0 + ln, h:h + 1])

                # ---- RoPE on qT, kT ----
                cs = cos_sb[:, sb * TOK:(sb + 1) * TOK]
                sn = sin_sb[:, sb * TOK:(sb + 1) * TOK]
                for dst in (qT, kT):
                    rot = qkpool.tile([HD, HEADS * TOK], BF16, tag="rot")
                    for c in range(3):
                        o = c * 32
                        nc.gpsimd.dma_start(rot[o:o + 16, :], dst[o + 16:o + 32, :])
                        nc.gpsimd.dma_start(rot[o + 16:o + 32, :], dst[o:o + 16, :])
                    for h in range(HEADS):
                        hs = slice(h * TOK, (h + 1) * TOK)
                        nc.vector.tensor_mul(rot[:, hs], rot[:, hs], sn)
                        nc.vector.tensor_mul(dst[:, hs], dst[:, hs], cs)
                        nc.vector.tensor_add(dst[:, hs], dst[:, hs], rot[:, hs])

                # ---- V projection into (tokens, 24*(96+1)) with ones col ----
                vsb = []
                for jc in range(4):
                    v = vpool.tile([128, HEADS * (HD + 1)], BF16)
                    ones = v[:].rearrange("p (h c) -> p h c", c=HD + 1)
                    nc.vector.memset(ones[:, :, HD:HD + 1], 1.0)
                    vsb.append(v)
                ncol = 0
                for n, w in enumerate(VN):
                    pvs = []
                    for jc in range(4):
                        pvs.append(psp.tile([128, 512], F32, tag="ps", name="vps"))
                    for kt in range(KT):
                        wvf = wvfpool.tile([128, 512], F32, tag="wvf")
                        nc.sync.dma_start(
                            wvf[:, 0:w],
                            w_d["wv"].ap()[kt * 128:(kt + 1) * 128,
                                           ncol:ncol + w])
                        wvb = wvbpool.tile([128, 512], BF16, tag="wvb")
                        nc.any.tensor_copy(wvb[:, 0:w], wvf[:, 0:w])
                        for jc in range(4):
                            nc.tensor.matmul(
                                pvs[jc][:, 0:w],
                                xb[kt][:, jc * 128:(jc + 1) * 128],
                                wvb[:, 0:w],
                                start=(kt == 0), stop=(kt == KT - 1))
                    h0 = ncol // HD
                    nh = w // HD
                    for jc in range(4):
                        dstv = vsb[jc][:].rearrange(
                            "p (h c) -> p h c", c=HD + 1)
                        nc.scalar.activation(
                            dstv[:, h0:h0 + nh, 0:HD],
                            pvs[jc][:, 0:w].rearrange(
                                "p (h c) -> p h c", c=HD),
                            AF.Copy)
                    ncol += w

                # ---- attention per head ----
                osb = [opool.tile([128, DIM], BF16, name="osb") for _ in range(4)]
                for h in range(HEADS):
                    hs = slice(h * TOK, (h + 1) * TOK)
                    ets = []
                    for jc in range(4):
                        st = psp.tile([128, TOK], F32, tag="ps")
                        nc.tensor.matmul(
                            st[:], kT[:, h * TOK + jc * 128: h * TOK + (jc + 1) * 128],
                            qT[:, hs], start=True, stop=True)
                        et = epool.tile([128, TOK], BF16)
                        nc.scalar.activation(et[:], st[:], AF.Exp, scale=scale)
                        ets.append(et)
                    pv = psp.tile([128, 4 * (HD + 1)], F32, tag="ps")
                    for ic in range(4):
                        for jc in range(4):
                            nc.tensor.matmul(
                                pv[:, ic * (HD + 1):(ic + 1) * (HD + 1)],
                                ets[jc][:, ic * 128:(ic + 1) * 128],
                                vsb[jc][:, h * (HD + 1):(h + 1) * (HD + 1)],
                                start=(jc == 0), stop=(jc == 3))
                    recip = spool.tile([128, 4], F32)
                    for ic in range(4):
                        nc.vector.reciprocal(
                            recip[:, ic:ic + 1],
                            pv[:, ic * (HD + 1) + HD: ic * (HD + 1) + HD + 1])
                    for ic in range(4):
                        nc.vector.tensor_scalar_mul(
                            osb[ic][:, h * HD:(h + 1) * HD],
                            pv[:, ic * (HD + 1): ic * (HD + 1) + HD],
                            recip[:, ic:ic + 1])

                # ---- transpose O (tok, dim) -> oT (dim, tok), into xb ----
                for mt in range(MT):
                    for ic in range(4):
                        trp = ptp.tile([128, 128], BF16)
                        nc.tensor.transpose(
                            trp[:], osb[ic][:, mt * 128:(mt + 1) * 128],
                            ident[:])
                        nc.any.tensor_copy(
                            xb[mt][:, ic * 128:(ic + 1) * 128], trp[:])

                # ---- output projection ----
                for m in range(MT):
                    wf = wfpool.tile([128, DIM], F32)
                    nc.sync.dma_start(
                        wf[:].rearrange("p (kt c) -> p kt c", kt=KT),
                        w_d["wo"].ap()[:, m * 128:(m + 1) * 128]
                        .rearrange("(kt p) c -> p kt c", p=128))
                    wb = wbpool.tile([128, DIM], BF16)
                    nc.any.tensor_copy(wb[:], wf[:])
                    ps = psp.tile([128, TOK], F32, tag="ps")
                    for kt in range(KT):
                        nc.tensor.matmul(
                            ps[:], wb[:, kt * 128:(kt + 1) * 128],
                            xb[kt][:], start=(kt == 0), stop=(kt == KT - 1))
                    of = ofpool.tile([128, TOK], F32)
                    nc.scalar.activation(of[:], ps[:], AF.Identity,
                                         bias=bo_sb[:, m:m + 1])
                    nc.sync.dma_start(
                        out_d.ap()[m * 128:(m + 1) * 128,
                                   sb * TOK:(sb + 1) * TOK], of[:])
    return nc


def _rope_tables(frame, height, width):
    """cos/sin (96, 2048) per global token s; sin has rotate-half sign
    absorbed (rows d with d%32<16 negated)."""
    s = np.arange(S)
    pos = np.stack([s // (height * width), (s // width) % height, s % width])
    d = np.arange(0, 32, 2, dtype=np.float32)
    inv = 1.0 / ROPE_FREQ ** (d / 32.0)          # (16,)
    cos = np.empty((HD, S), np.float32)
    sin = np.empty((HD, S), np.float32)
    for ax in range(3):
        f = pos[ax][None, :].astype(np.float32) * inv[:, None]   # (16, S)
        c, sn = np.cos(f), np.sin(f)
        for half in range(2):
            r = ax * 32 + half * 16
            cos[r:r + 16] = c
            sin[r:r + 16] = sn if half else -sn
    return cos, sin


_NC_CACHE = {}


def kernel(hidden_states, wq, bq, wk, bk, wv, bv, wo, bo,
           frame, height, width):
    frame, height, width = int(frame), int(height), int(width)
    hs = np.ascontiguousarray(hidden_states, dtype=np.float32)
    assert hs.shape == (S, B, DIM)

    if "nc" not in _NC_CACHE:
        _NC_CACHE["nc"] = build_nc()
    nc = _NC_CACHE["nc"]

    cos, sin = _rope_tables(frame, height, width)

    # sparse batch j = kk*B + b holds tokens s = g*NSP + kk
    hs_r = hs.reshape(SPS, NSP, B, DIM)          # (g, kk, b, dim)
    bq_r = np.ascontiguousarray(np.asarray(bq, np.float32).reshape(HEADS, HD).T)
    bk_r = np.ascontiguousarray(np.asarray(bk, np.float32).reshape(HEADS, HD).T)
    bo2 = np.asarray(bv, np.float32) @ np.asarray(wo, np.float32) \
        + np.asarray(bo, np.float32)
    bo_r = np.ascontiguousarray(bo2.reshape(MT, 128).T)
    wdict = {n: np.ascontiguousarray(a, np.float32)
             for n, a in (("wq", wq), ("wk", wk), ("wv", wv), ("wo", wo))}

    in_maps = []
    for c in range(NCORES):
        xs, css, sns = [], [], []
        for sbi in range(2):
            j = 2 * c + sbi
            kk, b = divmod(j, B)
            xs.append(hs_r[:, kk, b, :].T)                     # (dim, 512)
            tok = np.arange(SPS) * NSP + kk
            css.append(cos[:, tok])
            sns.append(sin[:, tok])
        in_maps.append({
            "x": np.ascontiguousarray(np.concatenate(xs, 1)),
            "cos": np.concatenate(css, 1).astype(ml_dtypes.bfloat16),
            "sin": np.concatenate(sns, 1).astype(ml_dtypes.bfloat16),
            "bq": bq_r, "bk": bk_r, "bo": bo_r, **wdict,
        })

    res = run_bass_kernel_spmd(nc, in_maps, list(range(NCORES)))

    out = np.empty((SPS, NSP, B, DIM), np.float32)
    for c in range(NCORES):
        o = res.results[c]["out"]                              # (dim, 1024)
        for sbi in range(2):
            j = 2 * c + sbi
            kk, b = divmod(j, B)
            out[:, kk, b, :] = o[:, sbi * TOK:(sbi + 1) * TOK].T
    return out.reshape(S, B, DIM)
